# revision 22
# baseline (speedup 1.0000x reference)
"""Multi-head attention (B=8, S=1024, D=2048, H=16) on 8 Trainium2 NeuronCores.

Sharding: pure data parallel — core b computes batch element b. Weights are
replicated. Inside each core everything runs in fp32r (TF32-like, ~1.5e-4 rel
err) on the tensor engine, softmax in fp32 on ACT/DVE.

Self-contained: builds the Bass program, shards inputs, runs SPMD via PJRT,
reassembles the full output.
"""
import numpy as np
from contextlib import ExitStack

import concourse.bacc as bacc
import concourse.mybir as mybir
import concourse.tile as tile
from concourse.masks import make_identity

B, S, D, H = 8, 1024, 2048, 16
DH = D // H            # 128
NK = D // 128          # 16 k-chunks
NS = S // 128          # 8 s-tiles
F32 = mybir.dt.float32
F32R = mybir.dt.float32r
SCALE = 1.0 / float(np.sqrt(DH))

_CACHE = {}


def round_f32r(x, drop=8):
    """Round-to-nearest-even dropping low `drop` mantissa bits (matches the
    precision the PE keeps for fp32r operands)."""
    b = np.ascontiguousarray(x, dtype=np.float32).view(np.uint32).astype(np.uint64)
    half = np.uint64(1 << (drop - 1))
    odd = (b >> np.uint64(drop)) & np.uint64(1)
    b = ((b + half - np.uint64(1) + odd) >> np.uint64(drop)) << np.uint64(drop)
    return b.astype(np.uint32).view(np.float32)


def _load_transposed(nc, pool_x, psum, xT, x_dram, ident, f32r_in=False):
    """x [S, D] fp32 in DRAM -> xT[j] [128, S] f32r SBUF tiles, via PE transpose.
    Evacuation copies ride on ACT (idle during projection phases)."""
    dt_in = F32R if f32r_in else F32
    for sh in range(2):                      # s-halves to bound SBUF
        xs = []
        for i in range(NS // 2):
            t = pool_x.tile([128, D], dt_in, name="xin")
            eng = nc.gpsimd if f32r_in else nc.sync
            eng.dma_start(t[:], x_dram[(sh * 4 + i) * 128:(sh * 4 + i + 1) * 128, :])
            xs.append(t)
        for j in range(NK):
            ps = psum.tile([128, 512], F32, name="ps512")
            psv = ps[:].bitcast(dt_in)
            for i in range(NS // 2):
                nc.tensor.transpose(psv[:, i * 128:(i + 1) * 128],
                                    xs[i][:, j * 128:(j + 1) * 128], ident[:])
            nc.scalar.copy(xT[j][:, sh * 512:(sh + 1) * 512], psv[:])


def build(opt=None, reps=1, timing=False):
    _defaults = dict(
        tails_deferred=1,    # emit both halves' mains before tails
        ps512_bufs=2,
        pssum_bufs=2,
        w_on_scalar=0,       # weight loads on the ACT HWDGE queue
        sc_bufs=3,           # scores ps1024 buffers
        wv_first=0,          # prefetch first Wv slices before transposes
        wv_split=2,          # split each Wv slice DMA for earlier compute start
        wq_split=2,          # split each Wq/Wk slice DMA likewise
        sw_pipe=1,           # software-pipeline scores/exp one chunk-pair ahead of PV
        hload_scalar=0,      # per-head qh/kh/vh loads on scalar HWDGE queue
        f32r_transpose=0,    # PE transposes at f32r (1.5 cyc/row) instead of fp32 (2)
        tail_shared_1024=1,  # tail rt+tr packed into one ps1024 slot
        osum_shared=1,       # ps_o and ps_s share the ps512 tag
        e_bufs=4,
        vout_bufs=4,
    )
    _defaults.update(opt or {})
    opt = _defaults
    nc = bacc.Bacc("TRN2", target_bir_lowering=False, debug=False)

    kind = "Internal" if timing else "ExternalInput"
    def _in(name, shape, dt_):
        if timing:
            return nc.dram_tensor(name, shape, dt_).ap()
        return nc.dram_tensor(name, shape, dt_, kind="ExternalInput").ap()

    q_d = _in("q", [S, D], F32)
    k_d = _in("k", [S, D], F32)
    v_d = _in("v", [S, D], F32)
    wq_d = _in("Wq", [D, D], F32R)
    wk_d = _in("Wk", [D, D], F32R)
    wv_d = _in("Wv", [D, D], F32R)
    if timing:
        out_d = nc.dram_tensor("out", [S, D], F32).ap()
        tout_d = nc.dram_tensor("tout", [1, 8], F32, kind="ExternalOutput").ap()
    else:
        out_d = nc.dram_tensor("out", [S, D], F32, kind="ExternalOutput").ap()

    vst = nc.dram_tensor("vst", [NS, 128, D], F32R)        # V[s,d] staged
    qst = nc.dram_tensor("qst", [H, 128, S], F32R)         # QT per head
    kst = nc.dram_tensor("kst", [H, 128, S], F32R)

    with tile.TileContext(nc) as tc, ExitStack() as ctx:
        psum = ctx.enter_context(tc.tile_pool(name="psum", bufs=opt["ps512_bufs"], space="PSUM"))
        const = ctx.enter_context(tc.tile_pool(name="const", bufs=1))
        pool_x = ctx.enter_context(tc.tile_pool(name="xin", bufs=6))

        ident = const.tile([128, 128], F32, name="ident")
        make_identity(nc, ident[:])
        ident_r = const.tile([128, 128], F32R, name="ident_r")
        nc.vector.tensor_copy(ident_r[:], ident[:])
        ones_f = const.tile([128, 1], F32, name="ones_f")
        nc.gpsimd.memset(ones_f[:], 1.0)
        ones = const.tile([128, 1], F32R, name="ones")
        nc.vector.tensor_copy(ones[:], ones_f[:])
        sspads = []
        sspad_dt = F32R if opt["f32r_transpose"] else F32
        for i in range(2):
            spf = const.tile([128, 512], F32, name=f"sspadf{i}")
            nc.gpsimd.memset(spf[:], 0.0)
            sp = const.tile([128, 512], sspad_dt, name=f"sspad{i}")
            nc.vector.tensor_copy(sp[:], spf[:])
            sspads.append(sp)

        if timing:
            # zero-fill the internal inputs once so exp() stays finite
            with tc.tile_pool(name="zfill", bufs=1) as zpool:
                zf = zpool.tile([128, D], F32, name="zfill")
                nc.gpsimd.memset(zf[:], 0.0)
                zr = zpool.tile([128, D], F32R, name="zfill_r")
                nc.vector.tensor_copy(zr[:], zf[:])
                for x in (q_d, k_d, v_d):
                    for i in range(NS):
                        nc.sync.dma_start(x[i * 128:(i + 1) * 128, :], zf[:])
                for w in (wq_d, wk_d, wv_d):
                    for i in range(NK):
                        nc.sync.dma_start(w[i * 128:(i + 1) * 128, :], zr[:])

        for _rep in range(reps):
            _body_once(nc, tc, psum, const, pool_x,
                       ident_r if opt["f32r_transpose"] else ident,
                       ones, sspads,
                       q_d, k_d, v_d, wq_d, wk_d, wv_d, out_d,
                       vst, qst, kst, opt)
        if timing:
            zo = const.tile([1, 8], F32, name="zo")
            nc.gpsimd.memset(zo[:], 0.0)
            nc.sync.dma_start(tout_d[:], zo[:])

    nc.compile()
    return nc


def _body_once(nc, tc, psum, const, pool_x, ident, ones, sspads,
               q_d, k_d, v_d, wq_d, wk_d, wv_d, out_d, vst, qst, kst, opt):
    if True:
        # ---------------- Phase V: v -> vT -> V -> vst ----------------
        with ExitStack() as pv:
            pool_vT = pv.enter_context(tc.tile_pool(name="vT", bufs=1))
            pool_w = pv.enter_context(tc.tile_pool(name="wv", bufs=2))
            wv_tiles = {}
            if opt["wv_first"]:
                for n in range(2):
                    wv_tiles[n] = pool_w.tile([128, NK, 512], F32R, name="wv")
                    nc.scalar.dma_start(
                        wv_tiles[n][:],
                        wv_d[:, n * 512:(n + 1) * 512].rearrange("(kc p) n -> p kc n", p=128))
            vT = [pool_vT.tile([128, S], F32R, name=f"vT{j}") for j in range(NK)]
            _load_transposed(nc, pool_x, psum, vT, v_d, ident, f32r_in=opt["f32r_transpose"])

            pool_o = pv.enter_context(tc.tile_pool(name="vout", bufs=opt["vout_bufs"]))
            for n in range(4):
                if n in wv_tiles:
                    wv_t = wv_tiles[n]
                else:
                    wv_t = pool_w.tile([128, NK, 512], F32R, name="wv")
                    src = wv_d[:, n * 512:(n + 1) * 512].rearrange("(kc p) n -> p kc n", p=128)
                    for qtr in range(opt.get("wv_split", 1)):
                        nq = NK // opt.get("wv_split", 1)
                        (nc.scalar if opt["w_on_scalar"] else nc.sync).dma_start(
                            wv_t[:, qtr * nq:(qtr + 1) * nq, :],
                            src[:, qtr * nq:(qtr + 1) * nq, :])
                for m in range(NS):
                    ps = psum.tile([128, 512], F32, name="ps512")
                    for jk in range(NK):
                        nc.tensor.matmul(ps[:], vT[jk][:, m * 128:(m + 1) * 128],
                                         wv_t[:, jk, :],
                                         start=(jk == 0), stop=(jk == NK - 1))
                    vo = pool_o.tile([128, 512], F32R, name="vout")
                    nc.vector.tensor_copy(vo[:], ps[:])
                    nc.sync.dma_start(vst.ap()[m, :, n * 512:(n + 1) * 512], vo[:])

        # ------------- Phases Q and K: x -> xT -> XT_h -> DRAM -------------
        for x_d, w_d, st in ((q_d, wq_d, qst), (k_d, wk_d, kst)):
            with ExitStack() as px:
                pool_xT = px.enter_context(tc.tile_pool(name="xT", bufs=1))
                xT = [pool_xT.tile([128, S], F32R, name=f"xT{j}") for j in range(NK)]
                _load_transposed(nc, pool_x, psum, xT, x_d, ident, f32r_in=opt["f32r_transpose"])

                pool_w = px.enter_context(tc.tile_pool(name="wq", bufs=2))
                pool_o = px.enter_context(tc.tile_pool(name="xtout", bufs=opt.get("xtout_bufs", 3)))
                for ws in range(8):            # 256-wide W slices = 2 heads each
                    w_t = pool_w.tile([128, NK, 256], F32R, name="wslice")
                    srcw = w_d[:, ws * 256:(ws + 1) * 256].rearrange("(kc p) n -> p kc n", p=128)
                    for qtr in range(opt.get("wq_split", 1)):
                        nq = NK // opt.get("wq_split", 1)
                        (nc.scalar if opt["w_on_scalar"] else nc.sync).dma_start(
                            w_t[:, qtr * nq:(qtr + 1) * nq, :],
                            srcw[:, qtr * nq:(qtr + 1) * nq, :])
                    for hl in range(2):
                        h = ws * 2 + hl
                        ps = psum.tile([128, 1024], F32, name="ps1024", bufs=opt["sc_bufs"])
                        for half in range(2):
                            for jk in range(NK):
                                nc.tensor.matmul(
                                    ps[:, half * 512:(half + 1) * 512],
                                    w_t[:, jk, hl * 128:(hl + 1) * 128],
                                    xT[jk][:, half * 512:(half + 1) * 512],
                                    start=(jk == 0), stop=(jk == NK - 1))
                        xo = pool_o.tile([128, S], F32R, name="xtout")
                        nc.vector.tensor_copy(xo[:], ps[:])
                        nc.sync.dma_start(st.ap()[h], xo[:])

        # ---------------- Phase heads: attention ----------------
        with ExitStack() as ph:
            pool_qh = ph.enter_context(tc.tile_pool(name="qh", bufs=3))
            pool_kh = ph.enter_context(tc.tile_pool(name="kh", bufs=3))
            pool_vh = ph.enter_context(tc.tile_pool(name="vh", bufs=2))
            pool_e = ph.enter_context(tc.tile_pool(name="e", bufs=opt["e_bufs"]))
            pool_ou = ph.enter_context(tc.tile_pool(name="ou", bufs=2))
            pool_os = ph.enter_context(tc.tile_pool(name="osb", bufs=2))
            pool_rs = ph.enter_context(tc.tile_pool(name="rs", bufs=2))

            for h in range(H):
                _hq = nc.scalar if opt["hload_scalar"] else nc.sync
                qh = pool_qh.tile([128, S], F32R, name="qh")
                _hq.dma_start(qh[:], qst.ap()[h])
                kh = pool_kh.tile([128, S], F32R, name="kh")
                _hq.dma_start(kh[:], kst.ap()[h])
                vh = pool_vh.tile([128, NS, 128], F32R, name="vh")
                _hq.dma_start(
                    vh[:], vst.ap()[:, :, h * 128:(h + 1) * 128].rearrange("m p d -> p m d"))

                # main loops for both halves first; tails after (keeps PE fed
                # while DVE/ACT evacuate)
                ps_os, ps_ss = [], []

                def _main(half):
                    ps_o = psum.tile([128, 512], F32, name="ps512")
                    if opt["osum_shared"]:
                        ps_s = psum.tile([1, 512], F32, name="ps512")
                    else:
                        ps_s = psum.tile([1, 512], F32, name="pssum", bufs=opt["pssum_bufs"])
                    ps_os.append(ps_o)
                    ps_ss.append(ps_s)

                    def _scores(cp):
                        ps_sc = psum.tile([128, 1024], F32, name="ps1024", bufs=opt["sc_bufs"])
                        for u in range(2):
                            c = cp * 2 + u
                            nc.tensor.matmul(
                                ps_sc[:, u * 512:(u + 1) * 512],
                                kh[:, c * 128:(c + 1) * 128],
                                qh[:, half * 512:(half + 1) * 512],
                                start=True, stop=True)
                        e_t = pool_e.tile([128, 1024], F32R, name="e")
                        nc.scalar.activation(e_t[:], ps_sc[:],
                                             mybir.ActivationFunctionType.Exp,
                                             scale=SCALE)
                        return e_t

                    def _pv(cp, e_t):
                        for u in range(2):
                            c = cp * 2 + u
                            nc.tensor.matmul(ps_o[:], vh[:, c, :],
                                             e_t[:, u * 512:(u + 1) * 512],
                                             start=(c == 0), stop=(c == NS - 1))
                            nc.tensor.matmul(ps_s[:], ones[:],
                                             e_t[:, u * 512:(u + 1) * 512],
                                             start=(c == 0), stop=(c == NS - 1))

                    if opt["sw_pipe"]:
                        e_prev = _scores(0)
                        for cp in range(1, 4):
                            e_cur = _scores(cp)
                            _pv(cp - 1, e_prev)
                            e_prev = e_cur
                        _pv(3, e_prev)
                    else:
                        for cp in range(4):
                            _pv(cp, _scores(cp))

                def _tail(half):
                    ps_o, ps_s = ps_os[half], ps_ss[half]
                    # transpose sums via zero-padded tile, then reciprocal
                    sspad = sspads[half]
                    nc.vector.tensor_copy(sspad[0:1, :], ps_s[:])
                    if opt["tail_shared_1024"]:
                        ps_tail = psum.tile([128, 1024], F32, name="ps1024",
                                            bufs=opt["sc_bufs"])
                        ps_rt = ps_tail[:, 0:512]
                    else:
                        ps_rt = psum.tile([128, 1024], F32, name="ps1024",
                                          bufs=opt["sc_bufs"])
                    ps_rt_v = ps_rt[:].bitcast(sspad.dtype) if sspad.dtype != F32 else ps_rt
                    for t in range(4):
                        nc.tensor.transpose(ps_rt_v[:, t * 128:(t + 1) * 128],
                                            sspad[:, t * 128:(t + 1) * 128], ident[:])
                    rs = pool_rs.tile([128, 4], F32, name="rs")
                    for t in range(4):
                        nc.vector.reciprocal(rs[:, t:t + 1], ps_rt[:, t * 128:t * 128 + 1])

                    # evacuate outT, transpose to natural layout, normalize
                    ou = pool_ou.tile([128, 512],
                                      F32R if opt["f32r_transpose"] else F32, name="ou")
                    nc.vector.tensor_copy(ou[:], ps_o[:])
                    if opt["tail_shared_1024"]:
                        ps_tr = ps_tail[:, 512:1024]
                    else:
                        ps_tr = psum.tile([128, 512], F32, name="ps512")
                    ps_tr_v = ps_tr[:].bitcast(ou.dtype) if ou.dtype != F32 else ps_tr
                    for t in range(4):
                        nc.tensor.transpose(ps_tr_v[:, t * 128:(t + 1) * 128],
                                            ou[:, t * 128:(t + 1) * 128], ident[:])
                    o_t = pool_os.tile([128, 4, 128], F32, name="osb")
                    for t in range(4):
                        nc.vector.tensor_scalar_mul(o_t[:, t, :],
                                                    ps_tr[:, t * 128:(t + 1) * 128],
                                                    rs[:, t:t + 1])
                    nc.sync.dma_start(
                        out_d[half * 512:(half + 1) * 512, h * 128:(h + 1) * 128]
                        .rearrange("(t p) d -> p t d", p=128),
                        o_t[:])

                if opt["tails_deferred"]:
                    _main(0); _main(1); _tail(0); _tail(1)
                else:
                    _main(0); _tail(0); _main(1); _tail(1)


def _make_runner(nc, n_cores):
    """Jitted SPMD runner (q/k/v sharded over cores, weights replicated)."""
    import jax
    from jax.sharding import Mesh, PartitionSpec
    from jax.experimental.shard_map import shard_map
    from concourse import bass2jax
    from concourse.bass2jax import _bass_exec_p, install_neuronx_cc_hook

    install_neuronx_cc_hook()
    partition_name = nc.partition_id_tensor.name if nc.partition_id_tensor else None
    in_names, out_names, out_avals, zero_outs = [], [], [], []
    for alloc in nc.m.functions[0].allocations:
        if not isinstance(alloc, mybir.MemoryLocationSet):
            continue
        name = alloc.memorylocations[0].name
        if alloc.kind == "ExternalInput":
            if name != partition_name:
                in_names.append(name)
        elif alloc.kind == "ExternalOutput":
            out_names.append(name)
            shape = tuple(alloc.tensor_shape)
            dtype = mybir.dt.np(alloc.dtype)
            out_avals.append(jax.core.ShapedArray(shape, dtype))
            zero_outs.append(np.zeros(shape, dtype))
    sharded_in = {"q", "k", "v"}
    n_params = len(in_names)
    in_names_all = in_names + out_names
    if partition_name is not None:
        in_names_all.append(partition_name)

    def _body(*args):
        operands = list(args)
        if partition_name is not None:
            operands.append(bass2jax.partition_id_tensor())
        outs = _bass_exec_p.bind(
            *operands,
            out_avals=tuple(out_avals),
            in_names=tuple(in_names_all),
            out_names=tuple(out_names),
            lowering_input_output_aliases=(),
            sim_require_finite=True,
            sim_require_nnan=True,
            nc=nc,
        )
        return tuple(outs)

    devices = jax.devices()[:n_cores]
    mesh = Mesh(np.asarray(devices), ("core",))
    in_specs = tuple(
        PartitionSpec("core") if n in sharded_in else PartitionSpec()
        for n in in_names
    ) + (PartitionSpec("core"),) * len(out_names)
    out_specs = (PartitionSpec("core"),) * len(out_names)
    jitted = jax.jit(
        shard_map(_body, mesh=mesh, in_specs=in_specs, out_specs=out_specs,
                  check_rep=False),
        keep_unused=True,
    )

    def run(shared_map_, per_core_maps):
        import jax as _jax
        args = []
        for n in in_names:
            if n in sharded_in:
                args.append(np.concatenate([m[n] for m in per_core_maps], axis=0))
            else:
                args.append(shared_map_[n])
        concat_zeros = [
            np.zeros((n_cores * z.shape[0], *z.shape[1:]), z.dtype) for z in zero_outs
        ]
        out_arrs = jitted(*args, *concat_zeros)
        _jax.block_until_ready(out_arrs)
        return [
            {
                name: np.asarray(out_arrs[i]).reshape(n_cores, *out_avals[i].shape)[c]
                for i, name in enumerate(out_names)
            }
            for c in range(n_cores)
        ]

    return run


def _get_compiled():
    if "run" not in _CACHE:
        nc = build()
        _CACHE["run"] = _make_runner(nc, B)
    return _CACHE["run"]


def kernel(q, k, v, Wq, Wk, Wv):
    run = _get_compiled()
    shared = {
        "Wq": round_f32r(np.asarray(Wq)),
        "Wk": round_f32r(np.asarray(Wk)),
        "Wv": round_f32r(np.asarray(Wv)),
    }
    q = np.asarray(q, dtype=np.float32)
    k = np.asarray(k, dtype=np.float32)
    v = np.asarray(v, dtype=np.float32)
    per_core = [{"q": q[b], "k": k[b], "v": v[b]} for b in range(B)]
    results = run(shared, per_core)
    out = np.stack([results[b]["out"] for b in range(B)], axis=0)
    return out.astype(np.float32)


if __name__ == "__main__":
    rng = np.random.default_rng(0)
    qq = rng.standard_normal((B, S, D), dtype=np.float32)
    kk = rng.standard_normal((B, S, D), dtype=np.float32)
    vv = rng.standard_normal((B, S, D), dtype=np.float32)
    sc = np.float32(1.0 / np.sqrt(D))
    Wq = rng.standard_normal((D, D), dtype=np.float32) * sc
    Wk = rng.standard_normal((D, D), dtype=np.float32) * sc
    Wv = rng.standard_normal((D, D), dtype=np.float32) * sc
    o = kernel(q=qq, k=kk, v=vv, Wq=Wq, Wk=Wk, Wv=Wv)
    print("out", o.shape, o.dtype, np.abs(o).max())
